# revision 5
# baseline (speedup 1.0000x reference)
"""MaxPool3d (kernel=3, stride=2, padding=1) on Trainium2, 8 NeuronCores.

Input  x: (2, 32, 128, 128, 128) f32  ->  Output: (2, 32, 64, 64, 64) f32.

Sharding: the 64 (b, c) slices are data-parallel; each of the 8 cores gets 8
slices, processed as 4 slice-pairs (a pair packs 2 slices into the 128 SBUF
partitions: partition 64*s + d//2 holds depth rows 2k/2k+1 of slice s in the
free-dim parity slot).

Per-core algorithm (separable max pooling H -> W -> D), fp16 intermediates
(max() commutes with the monotone f32->fp16 rounding, so the result equals
fp16(true max): rel err <= 2^-11; fp16 gives the DVE its 2x_1P perf mode on
every unit-stride max):
  - Hybrid loads, one chunk = hc+1 h-rows (one-row overlap), both parities:
    even-d rows stream via SWDGE (gpsimd) cast-DMA straight into fp16 SBUF;
    odd-d rows stream via HWDGE (sync ring) as f32 and ScalarE casts them.
    Two independent DMA paths keep HBM saturated.
  - H pool on the free h axis at 2x: F = max(r[2j], r[2j+1], r[2j+2]).
  - W pool on the free w axis (stride-2 sources, 1x): G = max over w window.
  - D pool across partitions: out = max(G_E, G_O, G_O shifted down one
    partition); the shift is a small SBUF->SBUF DMA; rows 0/64 are covered
    by an idempotent duplicate copy.
  - Output is stored as fp16 and upcast to f32 on the host.

The per-chunk tail work is software-pipelined: the partition-shift DMAs and
the final D-max of chunk c are emitted during chunk c+1, and the store during
chunk c+2, so no engine's in-order instruction stream ever blocks on a
late dependency (a stalled trigger would hold up the next chunk's load/cast).
Pair 0 uses small leading chunks so the DVE starts early.
"""

import os
import sys

sys.path.insert(0, "/opt/trn_rl_repo")

import numpy as np

# Shapes (hardcoded per problem spec)
B, C, D, H, W = 2, 32, 128, 128, 128
OD, OH, OW = 64, 64, 64
N_CORES = 8
SLICES_PER_CORE = (B * C) // N_CORES  # 8
PAIRS = SLICES_PER_CORE // 2  # 4
HC = 32  # max h rows pooled per chunk; tile holds HC+1 rows
CHUNKS_RAMP = [8, 24, 32, 32, 32]  # pair 0: DVE starts early
CHUNKS_STEADY = [32, 32, 32, 32]
assert sum(CHUNKS_RAMP) == H and max(CHUNKS_RAMP) == HC
assert sum(CHUNKS_STEADY) == H

_cache = {}


def _build():
    import concourse.mybir as mybir
    from concourse import bacc
    from concourse.tile import TileContext

    f32 = mybir.dt.float32
    f16 = mybir.dt.float16
    nc = bacc.Bacc()
    x_ext = nc.declare_dram_parameter(
        "x_shard", [SLICES_PER_CORE, D, H, W], f32, isOutput=False
    )
    y_ext = nc.declare_dram_parameter(
        "y_shard", [SLICES_PER_CORE, OD, OH, OW], f16, isOutput=True
    )

    with TileContext(nc) as tc:
        with (
            tc.tile_pool(name="xpool", bufs=3) as xpool,
            tc.tile_pool(name="hpool", bufs=3) as hpool,
            tc.tile_pool(name="fpool", bufs=2) as fpool,
            tc.tile_pool(name="gpool", bufs=3) as gpool,
            tc.tile_pool(name="ypool", bufs=3) as ypool,
        ):
            pending = []  # chunks awaiting shift+D2 (stage2) and store (stage3)

            def stage2(e):
                # partition-shift of the odd slab (scalar ring), then the
                # final D max.  Gs[k] = G_O[k-1]; rows 0/64 idempotent dup.
                Gt, Yh, ohc = e["Gt"], e["Yh"], e["ohc"]
                Gs = gpool.tile([128, 1, HC // 2, OW], f16, name="Gs", tag="Gs")
                nc.scalar.dma_start(
                    out=Gs[1:64, :, 0:ohc, :], in_=Gt[0:63, 1:2, 0:ohc, :]
                )
                nc.scalar.dma_start(
                    out=Gs[65:128, :, 0:ohc, :], in_=Gt[64:127, 1:2, 0:ohc, :]
                )
                nc.scalar.dma_start(
                    out=Gs[0:65:64, :, 0:ohc, :], in_=Gt[0:65:64, 1:2, 0:ohc, :]
                )
                nc.vector.tensor_max(
                    out=Yh[:, :, 0:ohc, :],
                    in0=Yh[:, :, 0:ohc, :],
                    in1=Gs[:, :, 0:ohc, :],
                )

            def stage3(e):
                nc.scalar.dma_start(
                    out=y_ext[
                        e["s0"] : e["s0"] + 2, :, e["oh0"] : e["oh0"] + e["ohc"], :
                    ],
                    in_=e["Yh"][:, :, 0 : e["ohc"], :],
                )

            for p in range(PAIRS):
                s0 = 2 * p
                sizes = CHUNKS_RAMP if p == 0 else CHUNKS_STEADY
                h0 = 0
                for c, hc in enumerate(sizes):
                    oh0 = h0 // 2
                    ohc = hc // 2
                    # ---- loads: E parity SWDGE cast, O parity HWDGE f32 ----
                    xh = hpool.tile([128, 2, HC + 1, W], f16, name="xh", tag="xh")
                    xo = xpool.tile([128, 1, HC + 1, W], f32, name="xo", tag="xo")
                    if c == 0:
                        nc.gpsimd.dma_start(
                            out=xh[:, 0:1, 1 : hc + 1, :],
                            in_=x_ext[s0 : s0 + 2, 0:D:2, 0:hc, :],
                        )
                        # h = -1 pad row: duplicate row 0 (max-idempotent)
                        nc.gpsimd.dma_start(
                            out=xh[:, 0:1, 0:1, :],
                            in_=x_ext[s0 : s0 + 2, 0:D:2, 0:1, :],
                        )
                        nc.sync.dma_start(
                            out=xo[:, :, 1 : hc + 1, :],
                            in_=x_ext[s0 : s0 + 2, 1:D:2, 0:hc, :],
                        )
                        nc.sync.dma_start(
                            out=xo[:, :, 0:1, :],
                            in_=x_ext[s0 : s0 + 2, 1:D:2, 0:1, :],
                        )
                    else:
                        nc.gpsimd.dma_start(
                            out=xh[:, 0:1, 0 : hc + 1, :],
                            in_=x_ext[s0 : s0 + 2, 0:D:2, h0 - 1 : h0 + hc, :],
                        )
                        nc.sync.dma_start(
                            out=xo[:, :, 0 : hc + 1, :],
                            in_=x_ext[s0 : s0 + 2, 1:D:2, h0 - 1 : h0 + hc, :],
                        )
                    # O parity cast (ScalarE)
                    nc.scalar.copy(
                        out=xh[:, 1:2, 0 : hc + 1, :], in_=xo[:, :, 0 : hc + 1, :]
                    )
                    # ---- H pool (free axis, fp16 2x): hc+1 rows -> hc/2 ----
                    Ft = fpool.tile([128, 2, HC // 2, W], f16, name="Ft", tag="Ft")
                    nc.vector.tensor_max(
                        out=Ft[:, :, 0:ohc, :],
                        in0=xh[:, :, 0:hc:2, :],
                        in1=xh[:, :, 1:hc:2, :],
                    )
                    nc.vector.tensor_max(
                        out=Ft[:, :, 0:ohc, :],
                        in0=Ft[:, :, 0:ohc, :],
                        in1=xh[:, :, 2 : hc + 1 : 2, :],
                    )
                    # ---- W pool (free axis, stride-2 sources, 1x) ----
                    Gt = gpool.tile([128, 2, HC // 2, OW], f16, name="Gt", tag="Gt")
                    nc.vector.tensor_max(
                        out=Gt[:, :, 0:ohc, :],
                        in0=Ft[:, :, 0:ohc, 0:W:2],
                        in1=Ft[:, :, 0:ohc, 1:W:2],
                    )
                    nc.vector.tensor_max(
                        out=Gt[:, :, 0:ohc, 1:OW],
                        in0=Gt[:, :, 0:ohc, 1:OW],
                        in1=Ft[:, :, 0:ohc, 1 : W - 2 : 2],
                    )
                    # ---- D pool part 1: unshifted max of the two slabs ----
                    Yh = ypool.tile([128, 1, HC // 2, OW], f16, name="Yh", tag="Yh")
                    nc.vector.tensor_max(
                        out=Yh[:, :, 0:ohc, :],
                        in0=Gt[:, 0:1, 0:ohc, :],
                        in1=Gt[:, 1:2, 0:ohc, :],
                    )
                    # ---- deferred tail work of earlier chunks ----
                    if len(pending) >= 1:
                        stage2(pending[-1])
                    if len(pending) >= 2:
                        stage3(pending[-2])
                    pending.append(
                        {"Gt": Gt, "Yh": Yh, "s0": s0, "oh0": oh0, "ohc": ohc}
                    )
                    h0 += hc
            # flush
            stage2(pending[-1])
            stage3(pending[-2])
            stage3(pending[-1])
    nc.compile()
    return nc


def _get_nc():
    if "nc" not in _cache:
        _cache["nc"] = _build()
    return _cache["nc"]


def run(x: np.ndarray, **spmd_kwargs):
    """Run the SPMD kernel; returns the BassKernelResults (for tracing)."""
    from concourse.bass_utils import run_bass_kernel_spmd

    nc = _get_nc()
    xs = np.ascontiguousarray(x, dtype=np.float32).reshape(B * C, D, H, W)
    in_maps = [
        {"x_shard": np.ascontiguousarray(xs[SLICES_PER_CORE * i : SLICES_PER_CORE * (i + 1)])}
        for i in range(N_CORES)
    ]
    return run_bass_kernel_spmd(nc, in_maps, list(range(N_CORES)), **spmd_kwargs)


def kernel(x: np.ndarray) -> np.ndarray:
    res = run(x)
    out = np.stack([res.results[i]["y_shard"] for i in range(N_CORES)])
    return out.reshape(B, C, OD, OH, OW).astype(np.float32)


# revision 6
# speedup vs baseline: 1.0326x; 1.0326x over previous
"""MaxPool3d (kernel=3, stride=2, padding=1) on Trainium2, 8 NeuronCores.

Input  x: (2, 32, 128, 128, 128) f32  ->  Output: (2, 32, 64, 64, 64) f32.

Sharding: the 64 (b, c) slices are data-parallel; each of the 8 cores gets 8
slices, processed as 4 slice-pairs (a pair packs 2 slices into the 128 SBUF
partitions: partition 64*s + d//2 holds depth rows 2k/2k+1 of slice s in the
free-dim parity slot).

Per-core algorithm (separable max pooling H -> W -> D), fp16 intermediates
(max() commutes with the monotone f32->fp16 rounding, so the result equals
fp16(true max): rel err <= 2^-11; fp16 gives the DVE its 2x_1P perf mode on
every unit-stride max):
  - SWDGE (gpsimd) cast-loads stream x f32 from HBM into fp16 SBUF tiles,
    hc+1 h-rows per chunk (one-row overlap).  hc=64 keeps HBM descriptors at
    33 KB contiguous runs and halves per-op overheads.
  - H pool on the free h axis at 2x: F = max(r[2j], r[2j+1], r[2j+2]).
  - W pool on the free w axis (stride-2 sources, 1x): G = max over w window.
  - D pool across partitions: out = max(G_E, G_O, G_O shifted down one
    partition); the shift is a small SBUF->SBUF DMA on the sync ring; rows
    0/64 are covered by an idempotent duplicate copy.
  - Output is stored as fp16 (scalar ring) and upcast to f32 on the host.

Engine roles keep every DMA-issuing stream single-purpose so the in-order
HWDGE/SWDGE FIFOs never block a load behind a late dependency: gpsimd =
loads only, sync = partition shifts only, scalar = stores only.  Pair 0 uses
small leading chunks so the DVE starts early.
"""

import os
import sys

sys.path.insert(0, "/opt/trn_rl_repo")

import numpy as np

# Shapes (hardcoded per problem spec)
B, C, D, H, W = 2, 32, 128, 128, 128
OD, OH, OW = 64, 64, 64
N_CORES = 8
SLICES_PER_CORE = (B * C) // N_CORES  # 8
PAIRS = SLICES_PER_CORE // 2  # 4
HC = 64  # max h rows pooled per chunk; tile holds HC+1 rows
CHUNKS_RAMP = [16, 48, 64]  # pair 0: DVE starts early
CHUNKS_STEADY = [64, 64]
assert sum(CHUNKS_RAMP) == H and max(CHUNKS_RAMP) == HC
assert sum(CHUNKS_STEADY) == H

_cache = {}


def _build():
    import concourse.mybir as mybir
    from concourse import bacc
    from concourse.tile import TileContext

    f32 = mybir.dt.float32
    f16 = mybir.dt.float16
    nc = bacc.Bacc()
    x_ext = nc.declare_dram_parameter(
        "x_shard", [SLICES_PER_CORE, D, H, W], f32, isOutput=False
    )
    y_ext = nc.declare_dram_parameter(
        "y_shard", [SLICES_PER_CORE, OD, OH, OW], f16, isOutput=True
    )

    with TileContext(nc) as tc:
        with (
            tc.tile_pool(name="hpool", bufs=3) as hpool,
            tc.tile_pool(name="fpool", bufs=2) as fpool,
            tc.tile_pool(name="gpool", bufs=2) as gpool,
            tc.tile_pool(name="ypool", bufs=2) as ypool,
        ):
            for p in range(PAIRS):
                s0 = 2 * p
                sizes = CHUNKS_RAMP if p == 0 else CHUNKS_STEADY
                h0 = 0
                for c, hc in enumerate(sizes):
                    oh0 = h0 // 2
                    ohc = hc // 2
                    # ---- SWDGE cast-load: hc+1 h rows, both parities ----
                    xh = hpool.tile([128, 2, HC + 1, W], f16, name="xh", tag="xh")
                    for par in (0, 1):
                        if c == 0:
                            nc.gpsimd.dma_start(
                                out=xh[:, par : par + 1, 1 : hc + 1, :],
                                in_=x_ext[s0 : s0 + 2, par : D : 2, 0:hc, :],
                            )
                            # h = -1 pad row: duplicate row 0 (max-idempotent)
                            nc.gpsimd.dma_start(
                                out=xh[:, par : par + 1, 0:1, :],
                                in_=x_ext[s0 : s0 + 2, par : D : 2, 0:1, :],
                            )
                        else:
                            nc.gpsimd.dma_start(
                                out=xh[:, par : par + 1, 0 : hc + 1, :],
                                in_=x_ext[s0 : s0 + 2, par : D : 2, h0 - 1 : h0 + hc, :],
                            )
                    # ---- H pool (free axis, fp16 2x): hc+1 rows -> hc/2 ----
                    Ft = fpool.tile([128, 2, HC // 2, W], f16, name="Ft", tag="Ft")
                    nc.vector.tensor_max(
                        out=Ft[:, :, 0:ohc, :],
                        in0=xh[:, :, 0:hc:2, :],
                        in1=xh[:, :, 1:hc:2, :],
                    )
                    nc.vector.tensor_max(
                        out=Ft[:, :, 0:ohc, :],
                        in0=Ft[:, :, 0:ohc, :],
                        in1=xh[:, :, 2 : hc + 1 : 2, :],
                    )
                    # ---- W pool (free axis, stride-2 sources, 1x) ----
                    Gt = gpool.tile([128, 2, HC // 2, OW], f16, name="Gt", tag="Gt")
                    nc.vector.tensor_max(
                        out=Gt[:, :, 0:ohc, :],
                        in0=Ft[:, :, 0:ohc, 0:W:2],
                        in1=Ft[:, :, 0:ohc, 1:W:2],
                    )
                    nc.vector.tensor_max(
                        out=Gt[:, :, 0:ohc, 1:OW],
                        in0=Gt[:, :, 0:ohc, 1:OW],
                        in1=Ft[:, :, 0:ohc, 1 : W - 2 : 2],
                    )
                    # ---- D pool (partition axis) ----
                    # shifted copy of the odd slab (sync ring): Gs[k] =
                    # G_O[k-1]; rows 0/64 get the idempotent unshifted value.
                    Gs = gpool.tile([128, 1, HC // 2, OW], f16, name="Gs", tag="Gs")
                    nc.sync.dma_start(
                        out=Gs[1:64, :, 0:ohc, :], in_=Gt[0:63, 1:2, 0:ohc, :]
                    )
                    nc.sync.dma_start(
                        out=Gs[65:128, :, 0:ohc, :], in_=Gt[64:127, 1:2, 0:ohc, :]
                    )
                    nc.sync.dma_start(
                        out=Gs[0:65:64, :, 0:ohc, :], in_=Gt[0:65:64, 1:2, 0:ohc, :]
                    )
                    Yh = ypool.tile([128, 1, HC // 2, OW], f16, name="Yh", tag="Yh")
                    nc.vector.tensor_max(
                        out=Yh[:, :, 0:ohc, :],
                        in0=Gt[:, 0:1, 0:ohc, :],
                        in1=Gt[:, 1:2, 0:ohc, :],
                    )
                    nc.vector.tensor_max(
                        out=Yh[:, :, 0:ohc, :],
                        in0=Yh[:, :, 0:ohc, :],
                        in1=Gs[:, :, 0:ohc, :],
                    )
                    # ---- store fp16 output rows (scalar ring) ----
                    nc.scalar.dma_start(
                        out=y_ext[s0 : s0 + 2, :, oh0 : oh0 + ohc, :],
                        in_=Yh[:, :, 0:ohc, :],
                    )
                    h0 += hc
    nc.compile()
    return nc


def _get_nc():
    if "nc" not in _cache:
        _cache["nc"] = _build()
    return _cache["nc"]


def run(x: np.ndarray, **spmd_kwargs):
    """Run the SPMD kernel; returns the BassKernelResults (for tracing)."""
    from concourse.bass_utils import run_bass_kernel_spmd

    nc = _get_nc()
    xs = np.ascontiguousarray(x, dtype=np.float32).reshape(B * C, D, H, W)
    in_maps = [
        {"x_shard": np.ascontiguousarray(xs[SLICES_PER_CORE * i : SLICES_PER_CORE * (i + 1)])}
        for i in range(N_CORES)
    ]
    return run_bass_kernel_spmd(nc, in_maps, list(range(N_CORES)), **spmd_kwargs)


def kernel(x: np.ndarray) -> np.ndarray:
    res = run(x)
    out = np.stack([res.results[i]["y_shard"] for i in range(N_CORES)])
    return out.reshape(B, C, OD, OH, OW).astype(np.float32)
